# revision 30
# baseline (speedup 1.0000x reference)
"""Cross-attention Trainium2 kernel (8 NeuronCores, data-parallel).

Problem: B=4, C=64, H=64, W=64.
  q = conv1x1(v1, wq, bq); k = conv1x1(v2, wk, bk); v = conv1x1(v2, wv, bv)
  tokens n = (c, h) pairs (N = C*H = 4096), feature dim = W = 64
  out = softmax(q @ k^T) @ v

Sharding: core i handles batch b = i//2 and the q-token half h in
[32*(i%2), 32*(i%2+1)).  Every core needs the full v2[b] (k/v side) but only
its h-slice of v1[b] (q side).  No collectives.

v3: transpose-free setup + parity block structure + warm clock + split exp.
  - host passes x WITH a ones row (bias via augmented contraction) and W
    pre-transposed WITH a bias row; q/k projections run with the x-chunk as
    the STATIONARY operand, so their output is already feature-major
    [w, token] -- the 96 PE transposes of the old setup are gone.
  - a projection chunk covers 2 h-rows: even-h tokens land on partitions
    0-63, odd-h on 64-127.  All main-loop blocks are parity-pure: qT2/kT2
    halves hold (E|O) tokens; qT2s (a DMA-swapped copy of qT2) provides the
    cross-parity score blocks.  vf blocks 0-15 are jE, 16-31 are jO.
  - PE warmup burst at kernel start trips the HAM clock gate to 8/8
    (2.4 GHz) before the projections.
  - scores are bf16 (1-pass matmuls, fp32 PSUM); projections are plain f32
    (2-pass, but the moving operand is only 64 cols).
  - exp split: 9/16 p-blocks on ScalarE (table exp), 7/16 on VectorE as a
    one-pass Schraudolph fast-exp (int16 = round(s*128*log2e + bias) whose
    bits ARE bf16(exp(s)) to ~3%; softmax normalization cancels most).
    The int16 write goes through a bitcast AP on the OUTPUT side -- a
    bitcast on a read operand breaks Tile dependency tracking (race).
  - softmax denominator via a ones-column appended to V; normalize on
    GpSimd after a PE transpose of the output accumulator.
"""

import numpy as np

B, C, H, W = 4, 64, 64, 64
HH = H // 2            # h-rows per core (q-token half)
NQ = C * HH            # q tokens per core = 2048
NK = C * H             # k tokens = 4096
NPB = 16               # j-block pairs (jE-block p, jO-block p), p = 0..15
IP = 256               # i-span per parity per pass (4 passes)
NCORES = 8

A_SCH = 128.0 * 1.4426950408889634
B_SCH = 127.0 * 128.0 - 0.0430 * 128.0

# p-blocks whose exp runs on VectorE (Schraudolph); rest on ScalarE
SCH_P = {1, 3, 5, 7, 9, 11, 13, 15}

_CACHE = {}


def _build_nc():
    from contextlib import ExitStack

    import concourse.bass as bass
    import concourse.tile as tile
    from concourse import bacc, mybir
    from concourse.bass import ts
    from concourse.masks import make_identity

    F32 = mybir.dt.float32
    BF16 = mybir.dt.bfloat16
    I16 = mybir.dt.int16
    AF = mybir.ActivationFunctionType
    ALU = mybir.AluOpType

    nc = bacc.Bacc(trn_type="TRN2", target_bir_lowering=False)

    # host-augmented inputs: x rows 0-63 = data, row 64 = 1.0;
    # wT rows 0-63 = W^T, row 64 = bias
    x1 = nc.declare_dram_parameter("x1", [C + 1, HH * W], F32, False)
    x2 = nc.declare_dram_parameter("x2", [C + 1, H * W], F32, False)
    wqT_d = nc.declare_dram_parameter("wqT", [C + 1, C], F32, False)
    wkT_d = nc.declare_dram_parameter("wkT", [C + 1, C], F32, False)
    wvT_d = nc.declare_dram_parameter("wvT", [C, C], F32, False)
    bv_d = nc.declare_dram_parameter("bv", [C, 1], F32, False)
    out_d = nc.declare_dram_parameter("out", [C, HH, W], F32, True)

    with ExitStack() as ctx:
        tc = ctx.enter_context(tile.TileContext(nc))
        cp = ctx.enter_context(tc.tile_pool(name="const", bufs=1))

        # inputs start moving immediately; small weight DMAs go FIRST so
        # the first Q-chunk isn't gated behind the big x transfers, and the
        # two initiator queues (sync, scalar) run x1 and x2 in parallel
        x1_st = cp.tile([C + 1, HH * W], F32)
        x2_st = cp.tile([C + 1, H * W], F32)
        wqT = cp.tile([C + 1, C], F32)
        wkT = cp.tile([C + 1, C], F32)
        wvT = cp.tile([C, C], F32)
        bv_sb = cp.tile([C, 1], F32)
        nc.scalar.dma_start(wqT[:, :], wqT_d[:, :])
        nc.scalar.dma_start(x1_st[:, 0:1024], x1[:, 0:1024])
        nc.sync.dma_start(x1_st[:, 1024:2048], x1[:, 1024:2048])
        nc.scalar.dma_start(wkT[:, :], wkT_d[:, :])
        nc.scalar.dma_start(wvT[:, :], wvT_d[:, :])
        nc.scalar.dma_start(bv_sb[:, :], bv_d[:, :])
        nc.scalar.dma_start(x2_st[:, :], x2[:, :])

        ident = cp.tile([128, 128], F32)
        make_identity(nc, ident[:, :])

        # bf16 copies for the V projection (V tolerates bf16 inputs; the
        # fp32 2-pass moving operand would cost 4x on the PE)
        x2bf = cp.tile([C, H * W], BF16)
        wvTb = cp.tile([C, C], BF16)
        nc.vector.tensor_copy(wvTb[:, :], wvT[:, :])

        # prewarm the exp table set while input DMAs run
        warm = cp.tile([128, 2], F32)
        nc.vector.memset(warm[:, :], 0.0)
        nc.scalar.activation(warm[:, 0:1], warm[:, 1:2], AF.Exp)

        # ---- PE warmup: contiguous matmul busy trips the HAM clock gate
        # to 8/8 (2.4 GHz) before the projections ----
        # warm-up / filler fuel: a zero bf16 tile whose LDWEIGHTS chains
        # keep the PE array busy (bubble-free, no PSUM) for the HAM gate
        wsb = cp.tile([128, 256], BF16)
        nc.gpsimd.memset(wsb[:, :], 0.0)
        with tc.tile_pool(name="wp0", bufs=1, space="PSUM") as wp0:
            wps0 = wp0.tile([128, 256], F32, tag="w0")
            for _ in range(34):
                nc.tensor.matmul(
                    wps0[:, :], lhsT=wsb[:, 0:128], rhs=wsb[:, :],
                    start=True, stop=True,
                )

        # qT2: [w, col=64*t+c] E q-tokens (c, 2t) on partitions 0-63,
        #      O q-tokens (c, 2t+1) on 64-127
        # qT2s: partition halves swapped (for cross-parity score blocks)
        # kT2: same layout over all 64 k h-pairs (2048 cols)
        qT2 = cp.tile([128, NQ // 2], BF16)
        qT2s = cp.tile([128, NQ // 2], BF16)
        kT2 = cp.tile([128, NK // 2], BF16)
        # vf: [idx, blk, 65]: blocks 0-15 = jE, 16-31 = jO; partition idx =
        # 64*t_rel + c = v-token (c, 2*(2p + t_rel) + parity); col 64 = 1.0
        vf = cp.tile([128, 32, 65], BF16)
        nc.gpsimd.memset(vf[:, :, 64:65], 1.0)

        _cp_n = [0]

        def psum_copy(dst, src, bias=None):
            e = _cp_n[0] % 2
            _cp_n[0] += 1
            if e == 0:
                if bias is None:
                    nc.scalar.activation(dst, src, AF.Copy)
                else:
                    nc.scalar.activation(dst, src, AF.Identity, bias=bias[:, 0:1])
            else:
                if bias is None:
                    nc.vector.tensor_copy(dst, src)
                else:
                    nc.vector.tensor_scalar(dst, src, bias[:, 0:1], None, ALU.add)

        with tc.tile_pool(name="pq", bufs=5, space="PSUM") as pq:
            def qk_chunk(dst, x_st, wT, t):
                # lhsT = x chunk [65, 128] (2 h-rows), rhs = wT_aug [65, 64]
                # out[p = 64*h1 + w, c] = proj[c, h = 2t+h1, w] + bias[c]
                ps = pq.tile([128, C], F32, tag="qk")
                nc.tensor.matmul(
                    ps[:, :], lhsT=x_st[:, ts(t, 128)], rhs=wT[:, :],
                    start=True, stop=True,
                )
                psum_copy(dst[:, ts(t, 64)], ps[:, :])

            # Q first, with the qT2s quarter swapped as soon as its four
            # chunks land (pass ih only needs quarter ih)
            for t in range(NQ // 128):
                qk_chunk(qT2, x1_st, wqT, t)
                if t % 4 == 3:
                    qq = (t // 4) * 256
                    nc.sync.dma_start(
                        qT2s[0:64, qq : qq + 256], qT2[64:128, qq : qq + 256]
                    )
                    nc.sync.dma_start(
                        qT2s[64:128, qq : qq + 256], qT2[0:64, qq : qq + 256]
                    )
                if t % 4 == 1:
                    # x2 bf16 casts early, split across engines
                    cc = (t // 4) * 1024
                    if t % 8 == 1:
                        nc.vector.tensor_copy(
                            x2bf[:, cc : cc + 1024], x2_st[0:C, cc : cc + 1024]
                        )
                    else:
                        nc.scalar.activation(
                            x2bf[:, cc : cc + 1024], x2_st[0:C, cc : cc + 1024],
                            AF.Copy,
                        )

        # K projections, serial (LDW-bound at the fixed NX clock, so the
        # HAM state barely matters here; keeping them out of the main loop
        # keeps the main loop pure-bf16, which is what trips the HAM warm)
        with tc.tile_pool(name="pqk", bufs=2, space="PSUM") as pqk:
            for p in range(NPB):
                ps = pqk.tile([128, 128], F32, tag="qk")
                for half in range(2):
                    nc.tensor.matmul(
                        ps[:, 64 * half : 64 * half + 64],
                        lhsT=x2_st[:, ts(2 * p + half, 128)],
                        rhs=wkT[:, :],
                        start=True, stop=True,
                    )
                psum_copy(kT2[:, ts(p, 128)], ps[:, :])

        # V projections (bf16 operands; vf scatter with parity blocks)
        with tc.tile_pool(name="pvV", bufs=2, space="PSUM") as pvV:
            for ch in range(8):
                ps = pvV.tile([C, 512], F32, tag="v")
                nc.tensor.matmul(
                    ps[:, :], lhsT=wvTb[:, :], rhs=x2bf[:, ts(ch, 512)],
                    start=True, stop=True,
                )
                pr = ps[:, :].rearrange(
                    "p (a b h1 w) -> p h1 b a w", a=2, b=2, h1=2, w=W
                )
                for h1 in range(2):
                    for b in range(2):
                        dst = vf[64 * b : 64 * b + 64,
                                 16 * h1 + 2 * ch : 16 * h1 + 2 * ch + 2, 0:W]
                        psum_copy(dst, pr[:, h1, b, :, :], bv_sb)

        # re-warm burst at main-loop entry: the setup's fp32/LDW mix does
        # not trip the HAM gate; ~8us of contiguous bf16 matmuls does, so
        # the main loop runs at 2.4 GHz from its first iteration
        with tc.tile_pool(name="wp2", bufs=1, space="PSUM") as wp2:
            wps2 = wp2.tile([128, 256], F32, tag="w2")
            for _ in range(40):
                nc.tensor.matmul(
                    wps2[:, :], lhsT=wsb[:, 0:128], rhs=wsb[:, :],
                    start=True, stop=True,
                )

        # ---- main attention loop (software-pipelined emission) ----
        # Global sequence gi = 16*ih + p with a 2-deep skew: scores+exp for
        # gi are emitted before PV for gi-2, so the in-order PE queue never
        # head-of-line blocks on an exp that is still running.  The K/V
        # setup chunks are fused into pass 0 (j-pair p only needs K-chunks
        # 2p,2p+1 and V-chunk p//2): their LDW/MM time fills the exp
        # latency bubbles of the early pipeline and keeps the HAM warm.
        outT_sb = cp.tile([C + 1, NQ], F32)

        def emit_scores_exp(sp, p_pool, ih, p):
            i0 = ih * IP
            sps = sp.tile([128, 4 * IP], F32, tag="scores")
            nc.tensor.matmul(
                sps[:, 0 * IP : 1 * IP],
                lhsT=kT2[0:64, ts(p, 128)],
                rhs=qT2[0:64, i0 : i0 + IP],
                start=True, stop=True,
            )
            nc.tensor.matmul(
                sps[:, 3 * IP : 4 * IP],
                lhsT=kT2[64:128, ts(p, 128)],
                rhs=qT2[64:128, i0 : i0 + IP],
                start=True, stop=True,
            )
            nc.tensor.matmul(
                sps[:, 1 * IP : 2 * IP],
                lhsT=kT2[0:64, ts(p, 128)],
                rhs=qT2s[0:64, i0 : i0 + IP],
                start=True, stop=True,
            )
            nc.tensor.matmul(
                sps[:, 2 * IP : 3 * IP],
                lhsT=kT2[64:128, ts(p, 128)],
                rhs=qT2s[64:128, i0 : i0 + IP],
                start=True, stop=True,
            )
            pt = p_pool.tile([128, 4 * IP], BF16, tag="p")
            if p in SCH_P:
                nc.vector.tensor_scalar(
                    pt[:, :].bitcast(I16), sps[:, :], A_SCH, B_SCH,
                    ALU.mult, ALU.add,
                )
            else:
                nc.scalar.activation(pt[:, :], sps[:, :], AF.Exp)
            return pt

        with (
            tc.tile_pool(name="outp", bufs=1, space="PSUM") as op_pool,
            tc.tile_pool(name="sp", bufs=3, space="PSUM") as sp,
            tc.tile_pool(name="tp2", bufs=1, space="PSUM") as tp2,
            tc.tile_pool(name="ppool", bufs=6) as p_pool,
            tc.tile_pool(name="opool", bufs=4) as o_pool,
            tc.tile_pool(name="rpool", bufs=4) as r_pool,
        ):
            pts = {}
            outT_acc = [None]

            def emit_drain(ih):
                i0 = ih * IP
                acc = accs[ih]
                if ih % 2 == 0:
                    nc.scalar.activation(
                        outT_sb[:, i0 : i0 + IP], acc[:, 0:IP], AF.Copy
                    )
                    nc.vector.tensor_copy(
                        outT_sb[:, 1024 + i0 : 1024 + i0 + IP],
                        acc[:, IP : 2 * IP],
                    )
                else:
                    nc.vector.tensor_copy(
                        outT_sb[:, i0 : i0 + IP], acc[:, 0:IP]
                    )
                    nc.scalar.activation(
                        outT_sb[:, 1024 + i0 : 1024 + i0 + IP],
                        acc[:, IP : 2 * IP], AF.Copy,
                    )

            def emit_tailblock(ih, par, tt):
                m = ih * (IP // 128) + tt
                col = 1024 * par + 128 * m
                psT = tp2.tile([128, C + 1], F32, tag="ot")
                nc.tensor.transpose(
                    psT[:, :], outT_sb[:, col : col + 128],
                    ident[0 : C + 1, 0 : C + 1],
                )
                rec = r_pool.tile([128, 1], F32, tag="rec")
                nc.vector.reciprocal(rec[:, :], psT[:, C : C + 1])
                ot = o_pool.tile([128, C], F32, tag="o")
                nc.scalar.activation(
                    ot[:, :], psT[:, 0:C], AF.Identity, scale=rec[:, 0:1]
                )
                dest = out_d[:, 4 * m : 4 * m + 4, :].rearrange(
                    "o (tr p2) w -> p2 tr o w", p2=2
                )[par]
                nc.sync.dma_start(dest, ot[:, :])

            accs = {}
            sched = {}
            for gi in range(64 + 14):
                ih, p = divmod(gi, 16)
                for item in sched.pop(gi, []):
                    emit_tailblock(*item)
                if gi < 64:
                    pts[gi] = emit_scores_exp(sp, p_pool, ih, p)
                if 2 <= gi < 66:
                    gj = gi - 2
                    jh, jp = divmod(gj, 16)
                    if jp == 0:
                        acc_t = op_pool.tile([C + 1, 2 * IP], F32, tag="outT")
                        accs[jh] = acc_t
                        outT_acc[0] = acc_t
                    pt = pts.pop(gj)
                    for par in range(2):
                        nc.tensor.matmul(
                            accs[jh][:, :],
                            lhsT=vf[:, 16 * par + jp, :],
                            rhs=pt[:, 2 * par * IP : (2 * par + 2) * IP],
                            start=(jp == 0 and par == 0),
                            stop=(jp == NPB - 1 and par == 1),
                        )
                    if jp == NPB - 1:
                        emit_drain(jh)
                        # spread the 4 normalize/store blocks over the next pass
                        for idx in range(4):
                            par, tt = divmod(idx, 2)
                            g = gi + 2 + 3 * idx
                            sched.setdefault(g, []).append((jh, par, tt))

    nc.compile()
    return nc


def _get_nc():
    if "nc" not in _CACHE:
        _CACHE["nc"] = _build_nc()
    return _CACHE["nc"]


def _in_maps(v1, v2, wq, bq, wk, bk, wv, bv):
    ones1 = np.ones((1, HH * W), np.float32)
    ones2 = np.ones((1, H * W), np.float32)
    wqT_aug = np.concatenate(
        [np.ascontiguousarray(wq, np.float32).T,
         np.asarray(bq, np.float32).reshape(1, C)], axis=0
    )
    wkT_aug = np.concatenate(
        [np.ascontiguousarray(wk, np.float32).T,
         np.asarray(bk, np.float32).reshape(1, C)], axis=0
    )
    maps = []
    for core in range(NCORES):
        b, half = divmod(core, 2)
        x1d = np.ascontiguousarray(
            v1[b, :, half * HH : (half + 1) * HH, :], dtype=np.float32
        ).reshape(C, HH * W)
        x2d = np.ascontiguousarray(v2[b], dtype=np.float32).reshape(C, H * W)
        maps.append({
            "x1": np.ascontiguousarray(np.concatenate([x1d, ones1], axis=0)),
            "x2": np.ascontiguousarray(np.concatenate([x2d, ones2], axis=0)),
            "wqT": np.ascontiguousarray(wqT_aug),
            "wkT": np.ascontiguousarray(wkT_aug),
            "wvT": np.ascontiguousarray(np.asarray(wv, np.float32).T),
            "bv": np.ascontiguousarray(np.asarray(bv, np.float32).reshape(C, 1)),
        })
    return maps


def _gather(results, v1):
    out = np.zeros((B, C, H, W), dtype=np.float32)
    for core in range(NCORES):
        b, half = divmod(core, 2)
        out[b, :, half * HH : (half + 1) * HH, :] = results[core]["out"]
    return out


def _run(trace=False, **inputs):
    from concourse.bass_utils import run_bass_kernel_spmd

    nc = _get_nc()
    maps = _in_maps(**inputs)
    res = run_bass_kernel_spmd(
        nc, maps, core_ids=list(range(NCORES)), trace=trace
    )
    return _gather(res.results, inputs["v1"]), res


def kernel(**inputs):
    out, _ = _run(trace=False, **inputs)
    return out


# revision 31
# speedup vs baseline: 1.0875x; 1.0875x over previous
"""Cross-attention Trainium2 kernel (8 NeuronCores, data-parallel).

Problem: B=4, C=64, H=64, W=64.
  q = conv1x1(v1, wq, bq); k = conv1x1(v2, wk, bk); v = conv1x1(v2, wv, bv)
  tokens n = (c, h) pairs (N = C*H = 4096), feature dim = W = 64
  out = softmax(q @ k^T) @ v

Sharding: core i handles batch b = i//2 and the q-token half h in
[32*(i%2), 32*(i%2+1)).  Every core needs the full v2[b] (k/v side) but only
its h-slice of v1[b] (q side).  No collectives.

Per-core algorithm (v2: warm-clock + bf16 scores + split exp):
  - PE warmup burst at kernel start trips the HAM clock gate to 8/8
    (2.4 GHz) before the projections; the whole kernel then runs warm.
  - everything on the matmul operand path is bf16 (scores matmuls are
    1-pass instead of fp32's 2-pass); accumulation stays fp32 in PSUM.
  - biases are folded into the PSUM->SBUF copies (ACT Identity+bias /
    DVE tensor_scalar add) instead of ones-row augmentation.
  - scores computed TRANSPOSED: sT[j, i] = k_j . q_i with k-tokens j on
    partitions; after exp the tile is exactly the stationary-operand layout
    the P@V matmul needs.  Two k-token blocks run concurrently in the PE
    via row groups (contraction is only W=64).
  - exp is split across engines: even p-blocks on ScalarE (table exp ->
    bf16), odd p-blocks on VectorE as a one-pass Schraudolph fast-exp:
    int16 = round(s * 128*log2e + (127*128 - 5.504)), whose bit pattern
    IS bf16(exp(s)) to ~3%; softmax normalization cancels most of it.
  - no max subtraction (|s| <= ~74 here; exp fits bf16's range); softmax
    denominator via a ones-column appended to V.
"""

import numpy as np

B, C, H, W = 4, 64, 64, 64
HH = H // 2            # h-rows per core (q-token half)
NQ = C * HH            # q tokens per core = 2048
NK = C * H             # k tokens = 4096
JB = NK // 128         # 32 j-blocks of 128 k-tokens
NP = JB // 2           # 16 row-packed j-block pairs
IP = 512               # i-span per pass (4 passes)
NCORES = 8

# Schraudolph fast-exp constants (bf16 bit pattern via int16):
# i16 = round(s * 128*log2e + 127*128 - 0.043*128)
A_SCH = 128.0 * 1.4426950408889634
B_SCH = 127.0 * 128.0 - 0.0430 * 128.0

USE_SCH = True       # odd p-blocks use DVE Schraudolph fast-exp
USE_BIAS_FOLD = True # biases via copy-stage (vs skipped entirely)
USE_WARMUP = True    # PE warmup burst

_CACHE = {}


def _build_nc():
    from contextlib import ExitStack

    import concourse.bass as bass
    import concourse.tile as tile
    from concourse import bacc, mybir
    from concourse.bass import ts
    from concourse.masks import make_identity

    F32 = mybir.dt.float32
    BF16 = mybir.dt.bfloat16
    F32R = mybir.dt.float32r
    I16 = mybir.dt.int16
    AF = mybir.ActivationFunctionType
    ALU = mybir.AluOpType

    nc = bacc.Bacc(trn_type="TRN2", target_bir_lowering=False)

    x1 = nc.declare_dram_parameter("x1", [C, HH * W], F32, False)
    x2 = nc.declare_dram_parameter("x2", [C, H * W], F32, False)
    wq_d = nc.declare_dram_parameter("wq", [C, C], F32, False)
    wk_d = nc.declare_dram_parameter("wk", [C, C], F32, False)
    wv_d = nc.declare_dram_parameter("wv", [C, C], F32, False)
    bq_d = nc.declare_dram_parameter("bq", [C, 1], F32, False)
    bk_d = nc.declare_dram_parameter("bk", [C, 1], F32, False)
    bv_d = nc.declare_dram_parameter("bv", [C, 1], F32, False)
    out_d = nc.declare_dram_parameter("out", [C, HH, W], F32, True)

    with ExitStack() as ctx:
        tc = ctx.enter_context(tile.TileContext(nc))
        cp = ctx.enter_context(tc.tile_pool(name="const", bufs=1))

        # inputs start moving immediately
        x1_st = cp.tile([C, HH * W], F32)
        x2_st = cp.tile([C, H * W], F32)
        w_sb = {}
        for name, wd in (("q", wq_d), ("k", wk_d), ("v", wv_d)):
            t = cp.tile([C, C], F32, tag=f"w_{name}")
            nc.scalar.dma_start(t[:, :], wd[:, :])
            w_sb[name] = t
        b_sb = {}
        for name, bd in (("q", bq_d), ("k", bk_d), ("v", bv_d)):
            t = cp.tile([C, 1], F32, tag=f"b_{name}")
            nc.scalar.dma_start(t[:, :], bd[:, :])
            b_sb[name] = t
        nc.scalar.dma_start(x1_st[:, 0:1024], x1[:, 0:1024])
        nc.sync.dma_start(x1_st[:, 1024:2048], x1[:, 1024:2048])
        nc.scalar.dma_start(x2_st[:, :], x2[:, :])

        ident = cp.tile([128, 128], F32)
        make_identity(nc, ident[:, :])

        # prewarm the exp table set while input DMAs run
        warm = cp.tile([128, 2], F32)
        nc.vector.memset(warm[:, :], 0.0)
        nc.scalar.activation(warm[:, 0:1], warm[:, 1:2], AF.Exp)

        # ---- PE warmup: ~6us of contiguous matmul busy trips the HAM
        # clock gate to 8/8 (2.4 GHz) before any real PE work ----
        if USE_WARMUP:
            ws = cp.tile([128, 256], BF16)
            nc.gpsimd.memset(ws[:, :], 0.0)
            with tc.tile_pool(name="wp", bufs=1, space="PSUM") as wp:
                wps = wp.tile([128, 256], F32, tag="warm")
                for _ in range(34):
                    nc.tensor.matmul(
                        wps[:, :], lhsT=ws[:, 0:128], rhs=ws[:, :],
                        start=True, stop=True,
                    )

        # bf16 operand staging for the projections
        x1_sb = cp.tile([C, HH * W], F32R)
        x2_sb = cp.tile([C, H * W], F32R)
        for c in range(HH * W // 1024):
            if c % 2:
                nc.scalar.activation(
                    x1_sb[:, ts(c, 1024)], x1_st[:, ts(c, 1024)], AF.Copy
                )
            else:
                nc.vector.tensor_copy(x1_sb[:, ts(c, 1024)], x1_st[:, ts(c, 1024)])
        for c in range(H * W // 1024):
            if c % 2:
                nc.scalar.activation(
                    x2_sb[:, ts(c, 1024)], x2_st[:, ts(c, 1024)], AF.Copy
                )
            else:
                nc.vector.tensor_copy(x2_sb[:, ts(c, 1024)], x2_st[:, ts(c, 1024)])

        # wT (bf16): rows = c_in, cols = c_out
        wT = {}
        with tc.tile_pool(name="pp0", bufs=2, space="PSUM") as pp0:
            for name in ("q", "k", "v"):
                t = cp.tile([C, C], F32R, tag=f"wT_{name}")
                ps = pp0.tile([C, C], F32, tag="wT_ps")
                nc.tensor.transpose(ps[:, :], w_sb[name][:, :], ident[0:C, 0:C])
                nc.vector.tensor_copy(t[:, :], ps[:, :])
                wT[name] = t

        # ---- projections (channel-major, bias folded) + transposes ----
        Q_cm = cp.tile([C, HH * W], F32)
        K_cm = cp.tile([C, H * W], F32)
        # qT2: (w, i) duplicated on both partition halves (rhs of scores)
        # kT2: (w, j) even j-blocks on partitions 0-63, odd on 64-127 (lhsT)
        qT2 = cp.tile([128, NQ], BF16)
        kT2 = cp.tile([128, NK // 2], BF16)

        # vf_aug (128, JB, 65) bf16: partition p of block jb = v-token
        # (c = p%64, h = 2*jb + p//64); col 64 = 1.0 (denominator trick)
        vf = cp.tile([128, JB, 65], BF16)
        nc.gpsimd.memset(vf[:, :, 64:65], 1.0)

        _cp_n = [0]

        def psum_copy(dst, src, bias=None, allow_act=True):
            if not USE_BIAS_FOLD:
                bias = None
            if allow_act and _cp_n[0] % 2 == 0:
                if bias is None:
                    nc.scalar.activation(dst, src, AF.Copy)
                else:
                    nc.scalar.activation(dst, src, AF.Identity, bias=bias[:, 0:1])
            else:
                if bias is None:
                    nc.vector.tensor_copy(dst, src)
                else:
                    nc.vector.tensor_scalar(
                        dst, src, bias[:, 0:1], None, ALU.add
                    )
            _cp_n[0] += 1

        with tc.tile_pool(name="pp1", bufs=3, space="PSUM") as pp1:
            def project(dst, wTt, x_sb, ch, bias, allow_act=False):
                ps = pp1.tile([C, 1024], F32, tag="setup")
                for c2 in range(2):
                    nc.tensor.matmul(
                        ps[:, ts(c2, 512)],
                        lhsT=wTt[:, :],
                        rhs=x_sb[:, ch * 1024 + c2 * 512 :][:, 0:512],
                        start=True, stop=True,
                    )
                psum_copy(dst[:, ts(ch, 1024)], ps[:, :], bias, allow_act)

            def project_v(ch, allow_act=False):
                # V: psum -> vf directly (bf16 cast + (h2,h1,w) rearrange)
                ps = pp1.tile([C, 1024], F32, tag="setup")
                for c2 in range(2):
                    nc.tensor.matmul(
                        ps[:, ts(c2, 512)],
                        lhsT=wT["v"][:, :],
                        rhs=x2_sb[:, ch * 1024 + c2 * 512 :][:, 0:512],
                        start=True, stop=True,
                    )
                pv = ps[:, :].rearrange("p (h2 h1 w) -> p h1 h2 w", h1=2, w=W)
                for h1 in range(2):
                    dst = vf[64 * h1 : 64 * (h1 + 1), ts(ch, 8), 0:W]
                    psum_copy(dst, pv[:, h1, :, :], b_sb["v"], allow_act)

            def q_transpose(grp, allow_act=False):
                ps = pp1.tile([64, 1024], F32, tag="setup")
                for hh in range(16):
                    h = grp * 16 + hh
                    nc.tensor.transpose(
                        ps[:, ts(hh, 64)], Q_cm[:, ts(h, 64)], ident[0:C, 0:C]
                    )
                psum_copy(qT2[0:64, ts(grp, 1024)], ps[:, :], None, allow_act)
                psum_copy(qT2[64:128, ts(grp, 1024)], ps[:, :], None, allow_act)

            def k_transpose(grp, allow_act=False):
                ps = pp1.tile([64, 1024], F32, tag="setup")
                for hh in range(16):
                    h = grp * 16 + hh
                    nc.tensor.transpose(
                        ps[:, ts(hh, 64)], K_cm[:, ts(h, 64)], ident[0:C, 0:C]
                    )
                pv = ps[:, :].rearrange("p (b two c) -> p b two c", two=2, c=128)
                for half in range(2):
                    dst = kT2[64 * half : 64 * half + 64, ts(grp, 512)].rearrange(
                        "p (b c) -> p b c", c=128
                    )
                    psum_copy(dst, pv[:, :, half, :], None, allow_act)

            # staggered emission: chunk g's transposes are emitted after
            # chunk g+1's projections so the in-order PE queue never waits
            # on the psum->sbuf copy of the chunk it just produced; pass 0
            # only needs qT2 group 0, so q_transpose(1) goes last
            for ch in range(HH * W // 1024):
                project(Q_cm, wT["q"], x1_sb, ch, b_sb["q"], allow_act=True)
            project(K_cm, wT["k"], x2_sb, 0, b_sb["k"], allow_act=True)
            project_v(0, allow_act=True)
            q_transpose(0, allow_act=True)
            project(K_cm, wT["k"], x2_sb, 1, b_sb["k"], allow_act=True)
            project_v(1, allow_act=True)
            k_transpose(0, allow_act=True)
            for ch in range(2, H * W // 1024):
                project(K_cm, wT["k"], x2_sb, ch, b_sb["k"], allow_act=True)
                project_v(ch, allow_act=True)
                k_transpose(ch - 1, allow_act=True)
            q_transpose(1, allow_act=True)
            k_transpose(H // 16 - 1, allow_act=True)

        # ---- main attention loop: 4 passes over i, row-packed j pairs ----
        # One PSUM tile per pair holds block A (cols 0-511) and block B
        # (cols 512-1023) at the same i-window: the two scores matmuls are
        # adjacent and overlap in the PE array (row groups 0-1 vs 2-3), and
        # a single FD=1024 exp covers both blocks.  exp alternates between
        # ScalarE (table exp) and VectorE (Schraudolph int16 fast-exp).
        outT_sb = cp.tile([C + 1, NQ], F32)
        with (
            tc.tile_pool(name="outp", bufs=1, space="PSUM") as op_pool,
            tc.tile_pool(name="sp", bufs=3, space="PSUM") as sp,
            tc.tile_pool(name="ppool", bufs=4) as p_pool,
            tc.tile_pool(name="tp2", bufs=1, space="PSUM") as tp2,
            tc.tile_pool(name="opool", bufs=4) as o_pool,
            tc.tile_pool(name="rpool", bufs=4) as r_pool,
        ):
            for ih in range(NQ // IP):
                i0 = ih * IP
                outT_ps = op_pool.tile([C + 1, IP], F32, tag="outT")
                pts2 = {}
                for p in range(NP + 2):
                    if p < NP:
                        sps = sp.tile([128, 2 * IP], F32, tag="scores")
                        for blk in range(2):
                            half = 64 * blk
                            nc.tensor.matmul(
                                sps[:, ts(blk, IP)],
                                lhsT=kT2[half : half + 64, ts(p, 128)],
                                rhs=qT2[half : half + 64, i0 : i0 + IP],
                                start=True, stop=True,
                            )
                        if p % 2 == 0 or not USE_SCH:
                            pt = p_pool.tile([128, 2 * IP], BF16, tag="p")
                            nc.scalar.activation(pt[:, :], sps[:, :], AF.Exp)
                        else:
                            pt = p_pool.tile([128, 2 * IP], BF16, tag="p")
                            nc.vector.tensor_scalar(
                                pt[:, :].bitcast(I16), sps[:, :], A_SCH, B_SCH,
                                ALU.mult, ALU.add,
                            )
                        pts2[p] = pt
                    if p >= 2:
                        q = p - 2
                        ptq = pts2.pop(q)
                        for blk in range(2):
                            jb = 2 * q + blk
                            nc.tensor.matmul(
                                outT_ps[:, :],
                                lhsT=vf[:, jb, :],
                                rhs=ptq[:, ts(blk, IP)],
                                start=(q == 0 and blk == 0),
                                stop=(q == NP - 1 and blk == 1),
                            )
                # drain this pass's accumulator to SBUF, then normalize +
                # store its four output tiles while the next pass runs
                dst = outT_sb[:, i0 : i0 + IP]
                if ih % 2 == 0:
                    nc.scalar.activation(dst, outT_ps[:, :], AF.Copy)
                else:
                    nc.vector.tensor_copy(dst, outT_ps[:, :])
                for tt in range(IP // 128):
                    t = ih * (IP // 128) + tt
                    ps = tp2.tile([128, C + 1], F32, tag="ot")
                    nc.tensor.transpose(
                        ps[:, :], outT_sb[:, ts(t, 128)], ident[0 : C + 1, 0 : C + 1]
                    )
                    rec = r_pool.tile([128, 1], F32, tag="rec")
                    nc.vector.reciprocal(rec[:, :], ps[:, C : C + 1])
                    ot = o_pool.tile([128, C], F32, tag="o")
                    nc.vector.tensor_scalar_mul(ot[:, :], ps[:, 0:C], rec[:, 0:1])
                    # rows p = h_loc*64 + o  ->  out[o, 2t + h_loc, :]
                    dest = out_d[:, 2 * t : 2 * t + 2, :].rearrange("o h w -> h o w")
                    nc.sync.dma_start(dest, ot[:, :])

    nc.compile()
    return nc


def _get_nc():
    if "nc" not in _CACHE:
        _CACHE["nc"] = _build_nc()
    return _CACHE["nc"]


def _in_maps(v1, v2, wq, bq, wk, bk, wv, bv):
    maps = []
    for core in range(NCORES):
        b, half = divmod(core, 2)
        maps.append({
            "x1": np.ascontiguousarray(
                v1[b, :, half * HH : (half + 1) * HH, :], dtype=np.float32
            ).reshape(C, HH * W),
            "x2": np.ascontiguousarray(v2[b], dtype=np.float32).reshape(C, H * W),
            "wq": np.ascontiguousarray(wq, dtype=np.float32),
            "wk": np.ascontiguousarray(wk, dtype=np.float32),
            "wv": np.ascontiguousarray(wv, dtype=np.float32),
            "bq": np.ascontiguousarray(bq, dtype=np.float32).reshape(C, 1),
            "bk": np.ascontiguousarray(bk, dtype=np.float32).reshape(C, 1),
            "bv": np.ascontiguousarray(bv, dtype=np.float32).reshape(C, 1),
        })
    return maps


def _gather(results, v1):
    out = np.zeros((B, C, H, W), dtype=np.float32)
    for core in range(NCORES):
        b, half = divmod(core, 2)
        out[b, :, half * HH : (half + 1) * HH, :] = results[core]["out"]
    return out


def _run(trace=False, **inputs):
    from concourse.bass_utils import run_bass_kernel_spmd

    nc = _get_nc()
    maps = _in_maps(**inputs)
    res = run_bass_kernel_spmd(
        nc, maps, core_ids=list(range(NCORES)), trace=trace
    )
    return _gather(res.results, inputs["v1"]), res


def kernel(**inputs):
    out, _ = _run(trace=False, **inputs)
    return out
